# revision 25
# baseline (speedup 1.0000x reference)
"""Trainium2 Bass kernel for nn_CapsuleLayer (dynamic routing capsule layer).

Reference computation:
    u_hat = einsum('jidk,bik->bjid', W, inputs)        # [B,J,I,D]
    b = 0
    for r in 0..2:
        c = softmax_j(b)                               # [B,J,1,I]
        s = einsum('bjoi,bjid->bjod', c, u_hat)        # [B,J,1,D]
        out = squash(s)
        if r < 2: b += einsum('bjod,bjid->bjoi', out, u_hat)
    return out                                         # [B,J,D]

Strategy: shard I (=2048) across 8 cores (I_loc=256), keep full B=128 on
every core (so PE matmuls stream N=128..256).  u_hat (168 MB) is NEVER
materialized: both routing contractions are expressed against W directly:

    s[b,j,d]  = sum_{i,k} (c[b,j,i] * x[b,i,k]) * W[j,i,d,k]   (PE, K=(k,i))
    a[b,j,i]  = sum_k x[b,i,k] * T[b,j,k,i],
    T[b,j,k,i] = sum_d out[b,j,d] * W[j,i,d,k]                 (PE, K=d)

One 80 KB AllReduce per routing iteration combines the per-core s partial
sums.  Softmax over J is local to each i (no cross-core traffic).

Matmul operands are stored in bf16 (fp32 matmuls on TRN2 cost a 2-way
hi/lo split plus a slow 4-byte LDWEIGHTS; bf16 gets fast-weight-load and
1 cycle/row).  All accumulation stays fp32 (PSUM / DVE internal).
"""

import numpy as np
from contextlib import ExitStack

import concourse.bass as bass
import concourse.bacc as bacc
import concourse.tile as tile
from concourse import mybir
from concourse.bass_utils import run_bass_kernel_spmd
from concourse.masks import make_identity

F32 = mybir.dt.float32
BF16 = mybir.dt.float16  # fp16: 10-bit mantissa, same PE/DVE speed class as bf16
AX = mybir.AxisListType
OP = mybir.AluOpType
ACTF = mybir.ActivationFunctionType

B = 128       # batch
I = 2048      # input capsules (sharded)
K = 8         # DIN
J = 10        # output capsules
D = 16        # DOUT
R = 3         # routing iterations
NCORES = 8

K_EPS = 1e-7
NORM_EPS = 1e-6


def bcast(ap: bass.AP, n: int) -> bass.AP:
    """Append a stride-0 (broadcast) innermost free dim of size n."""
    return bass.AP(ap.tensor, ap.offset, [*ap.ap, [0, n]])


def build_nc(n_cores: int = NCORES):
    IL = I // NCORES          # 256 per-core input capsules (also for n_cores=1 sim)
    IT = IL // 128            # 2 partition tiles of i
    NCH = IL * K // 128       # 16 (k,i)-chunks of 128 contraction rows
    KH = K // 2               # T computed in two 4-k halves (PSUM budget)

    nc = bacc.Bacc(num_devices=n_cores)

    x_ext = nc.dram_tensor("x", [B, IL, K], F32, kind="ExternalInput")
    w_ext = nc.dram_tensor("w", [J, IL, D, K], F32, kind="ExternalInput")
    out_ext = nc.dram_tensor("out", [B, J, D], F32, kind="ExternalOutput")
    ar_in = nc.dram_tensor("ar_in", [B, J * D], F32)
    ar_out = nc.dram_tensor("ar_out", [B, J * D], F32, addr_space="Shared")

    with tile.TileContext(nc) as tc, ExitStack() as ctx:
        sb = ctx.enter_context(tc.tile_pool(name="sb", bufs=1))
        ypool = ctx.enter_context(tc.tile_pool(name="ypool", bufs=2))
        pst = ctx.enter_context(tc.tile_pool(name="pst", bufs=2, space="PSUM"))
        ps_s_pool = ctx.enter_context(tc.tile_pool(name="ps_s", bufs=1, space="PSUM"))
        ps_sT_pool = ctx.enter_context(tc.tile_pool(name="ps_sT", bufs=2, space="PSUM"))
        ps_t_pool = ctx.enter_context(tc.tile_pool(name="ps_t", bufs=2, space="PSUM"))

        ident = sb.tile([128, 128], F32)
        make_identity(nc, ident)

        # persistent tensors used throughout
        x_kc = sb.tile([128, K, IL], F32)       # x[b, k, i] (k-outer) fp32
        x_kc_bf = sb.tile([128, K, IL], BF16)   # bf16 copy for the a-phase mul
        x_t = sb.tile([128, NCH, 128], BF16)    # x^T: [(i%128), (k,it), b]
        w_nat = sb.tile([128, IT, J, D * K], F32)   # w[i%128, it, j, (d,k)]
        w_bf = sb.tile([128, IT, J, D * K], BF16)   # bf16 copy (s-matmul rhs)
        w_kd_k = sb.tile([16, K, J, IT, 128], BF16)  # w[d, k, j, it, i] (base-0 rows)
        w_kd_bf = sb.tile([128, J, IT, 128], BF16)   # staging for the re-base

        # ---------------- persistent routing state ----------------
        a_t = sb.tile([128, IL, J], F32)      # agreements (i-outer, j-inner)
        e_t = sb.tile([128, IL, J], F32)      # exp(b_logits)
        z_t = sb.tile([128, IL], F32)         # softmax denominator
        rz_t = sb.tile([128, IL], F32)
        rz_scratch = sb.tile([128, IL], F32)
        c_t = sb.tile([128, IL, J], F32)      # routing weights (reused as exp scratch)
        cT_t = sb.tile([128, IT, J, 128], BF16)  # c transposed: [(i%128), it, j, b]
        s_sb = sb.tile([128, J, D], F32)      # all-reduced s
        s_stage = sb.tile([128, J, D], F32)   # pre-allreduce staging (iter 0)
        sT_stage = sb.tile([16, J, 128], F32)  # pre-allreduce staging (iters 1+)
        sT_sb = sb.tile([16, J, 128], F32)    # all-reduced sT
        out_sb = sb.tile([128, J, D], F32)    # squash output
        outT = sb.tile([16, J, 128], BF16)    # out transposed: [d, j, b]
        t_sb = sb.tile([128, K, IL], BF16)    # T copied out of PSUM (per j)
        p_big = sb.tile([128, K, IL], BF16)   # x * T elementwise
        tr1 = sb.tile([128, K // 2, IL], BF16)  # reduction tree temps
        tr2 = sb.tile([128, K // 4, IL], BF16)

        # small squash temps
        sq_s2 = sb.tile([128, J], F32)
        sq_mean = sb.tile([128, J], F32)
        sq_t = sb.tile([128, J, D], F32)
        sq_var = sb.tile([128, J], F32)
        sq_ln = sb.tile([128, J], F32)
        sq_rs = sb.tile([128, J], F32)
        sq_u = sb.tile([128, J], F32)
        sq_den = sb.tile([128, J], F32)
        sq_rden = sb.tile([128, J], F32)
        sq_scale = sb.tile([128, J], F32)
        sq_m2 = sb.tile([128, J], F32)
        eps_k = sb.tile([128, 1], F32)
        nc.vector.memset(eps_k[:], K_EPS)
        eps_n = sb.tile([128, 1], F32)
        nc.vector.memset(eps_n[:], NORM_EPS)

        # ---------------- load + relayout ----------------
        # Setup scratch aliases routing-state tensors that are not live yet
        # (a_t / e_t / c_t); Tile's dependency tracking orders the reuse.
        x_nat = bass.AP(a_t.tensor, a_t[:].offset, [a_t[:].ap[0], [K, IL], [1, K]])
        nc.sync.dma_start(out=x_nat, in_=x_ext[:])
        nc.vector.tensor_copy(
            out=x_kc[:],
            in_=bass.AP(a_t.tensor, a_t[:].offset, [a_t[:].ap[0], [1, K], [K, IL]]),
        )
        nc.vector.tensor_copy(out=x_kc_bf[:], in_=x_kc[:])
        for chh in range(NCH):
            k, it = divmod(chh, IT)
            p = pst.tile([128, 128], F32, tag="tr")
            nc.tensor.transpose(p[:], x_kc[:, k, it * 128:(it + 1) * 128], ident[:])
            nc.scalar.copy(out=x_t[:, chh, :], in_=p[:])

        for it in range(IT):
            nc.sync.dma_start(
                out=w_nat[:, it, :, :],
                in_=w_ext.rearrange("j (it p) d k -> it p j (d k)", it=IT)[it],
            )
        nc.vector.tensor_copy(out=w_bf[:], in_=w_nat[:])
        # shuffle (d,k) -> (k,d), transpose to [(k,d), j, it, i], then
        # re-base each k's 16 rows to partition 0 via SBUF->SBUF DMA.
        w_nat2 = bass.AP(
            e_t.tensor, e_t[:].offset,
            [e_t[:].ap[0], [J * K * D, IT], [K * D, J], [D, K], [1, D]])
        nc.vector.tensor_copy(
            out=w_nat2,
            in_=w_nat.rearrange("p it j (d k) -> p it j k d", k=K),
        )
        for it in range(IT):
            for j in range(J):
                p = pst.tile([128, 128], F32, tag="tr")
                nc.tensor.transpose(
                    p[:],
                    bass.AP(e_t.tensor,
                            e_t[:].offset + (it * J + j) * K * D,
                            [e_t[:].ap[0], [1, K * D]]),
                    ident[:],
                )
                nc.scalar.copy(out=w_kd_bf[:, j, it, :], in_=p[:])
        for k in range(K):
            nc.sync.dma_start(
                out=w_kd_k[:, k, :, :, :],
                in_=w_kd_bf[k * 16:(k + 1) * 16, :, :, :],
            )

        def w_rhs(it: int, j: int, k: int) -> bass.AP:
            """bf16 W slice [(i%128) x d] with d strided over the (d,k) dim."""
            return w_bf[:, it, j, :].rearrange("p (d k) -> p k d", k=K)[:, k, :]

        for r in range(R):
            # ---------- s matmuls ----------
            if r == 0:
                # c uniform 1/J: s_raw = sum_i u_hat.  x^T is j-independent,
                # so batch all (j,d) into one N=160 stream per chunk.
                ps_s = ps_s_pool.tile([128, J, D], F32, tag="s")
                for chh in range(NCH):
                    k, it = divmod(chh, IT)
                    rhs_all = w_bf[:, it, :, :].rearrange(
                        "p j (d k) -> p k j d", k=K)[:, k, :, :]
                    nc.tensor.matmul(
                        ps_s[:], lhsT=x_t[:, chh, :],
                        rhs=rhs_all.rearrange("p j d -> p (j d)"),
                        start=(chh == 0), stop=(chh == NCH - 1),
                    )
                nc.scalar.mul(out=s_stage[:], in_=ps_s[:], mul=1.0 / J)
                nc.sync.dma_start(
                    out=ar_in[:], in_=s_stage.rearrange("b j d -> b (j d)"))
            else:
                # stationary = W slices (16-col weight loads); moving = Y
                # (N=128).  Output lands transposed (sT[d, b] per j) and is
                # all-reduced in that layout (elementwise add is layout-
                # agnostic); only the reduced result is transposed back.
                for j in range(J):
                    # Y_j[(i%128), ch, b] = cT[(i%128), it, j, b] * x_t[:, ch, b]
                    y_j = ypool.tile([128, NCH, 128], BF16, tag="y")
                    cT_b = bass.AP(
                        cT_t.tensor, cT_t[:, 0, j, :].offset,
                        [cT_t.ap[0], [0, K], cT_t.ap[1], cT_t.ap[3]],
                    )  # dims [p, k(bcast), it, b]
                    nc.vector.tensor_tensor(
                        out=y_j[:], in0=x_t[:], in1=cT_b, op=OP.mult,
                    )
                    ps_sT = ps_sT_pool.tile([16, 128], F32, tag="sT")
                    for chh in range(NCH):
                        k, it = divmod(chh, IT)
                        nc.tensor.matmul(
                            ps_sT[:], lhsT=w_rhs(it, j, k), rhs=y_j[:, chh, :],
                            start=(chh == 0), stop=(chh == NCH - 1),
                        )
                    nc.scalar.copy(out=sT_stage[:, j, :], in_=ps_sT[:])
                nc.sync.dma_start(
                    out=ar_in.rearrange("b f -> (b f)").rearrange(
                        "(p f) -> p f", p=16),
                    in_=sT_stage.rearrange("d j b -> d (j b)"))

            # ---------- all-reduce s ----------
            if n_cores > 1:
                nc.gpsimd.collective_compute(
                    "AllReduce", OP.add,
                    replica_groups=[list(range(n_cores))],
                    ins=[ar_in[:]], outs=[ar_out[:]],
                )
                ar_res = ar_out
            else:
                ar_res = ar_in

            if r == 0:
                nc.sync.dma_start(
                    out=s_sb.rearrange("b j d -> b (j d)"), in_=ar_res[:])
            else:
                nc.sync.dma_start(
                    out=sT_sb.rearrange("d j b -> d (j b)"),
                    in_=ar_res.rearrange("b f -> (b f)").rearrange(
                        "(p f) -> p f", p=16))
                for j in range(J):
                    p = pst.tile([128, 128], F32, tag="tr")
                    nc.tensor.transpose(
                        p[:, :16], sT_sb[:, j, :], ident[:16, :16])
                    nc.scalar.copy(out=s_sb[:, j, :], in_=p[:, :16])

            # ---------- squash ----------
            v = s_sb  # [128, J, D]
            # s2 = sum_d (v/5)^2 ; mean = sum_d v / D
            nc.vector.scalar_tensor_tensor(
                out=sq_t[:], in0=v[:], scalar=0.04, in1=v[:],
                op0=OP.mult, op1=OP.mult)
            nc.vector.reduce_sum(out=sq_s2[:], in_=sq_t[:], axis=AX.X)
            nc.vector.reduce_sum(out=sq_mean[:], in_=v[:], axis=AX.X)
            nc.vector.tensor_scalar_mul(sq_mean[:], sq_mean[:], 1.0 / D)
            # t = v - mean ; var = sum_d t^2 / D
            nc.vector.tensor_sub(sq_t[:], v[:], bcast(sq_mean[:], D))
            nc.vector.tensor_tensor(out=v[:], in0=sq_t[:], in1=sq_t[:], op=OP.mult)
            nc.vector.reduce_sum(out=sq_var[:], in_=v[:], axis=AX.X)
            nc.vector.tensor_scalar_mul(sq_var[:], sq_var[:], 1.0 / D)
            # rs = 1/sqrt(s2 + K_EPS) = exp(-0.5*ln(s2 + K_EPS))
            nc.scalar.activation(out=sq_ln[:], in_=sq_s2[:], func=ACTF.Ln, bias=eps_k[:])
            nc.scalar.activation(out=sq_rs[:], in_=sq_ln[:], func=ACTF.Exp, scale=-0.5)
            # scale = 0.5*s2/(1+0.5*s2) * rs
            nc.vector.tensor_scalar_mul(sq_u[:], sq_s2[:], 0.5)
            nc.vector.tensor_scalar_add(sq_den[:], sq_u[:], 1.0)
            nc.vector.reciprocal(out=sq_rden[:], in_=sq_den[:])
            nc.vector.tensor_tensor(out=sq_scale[:], in0=sq_u[:], in1=sq_rden[:], op=OP.mult)
            nc.vector.tensor_tensor(out=sq_scale[:], in0=sq_scale[:], in1=sq_rs[:], op=OP.mult)
            # rvar = 1/sqrt(var + NORM_EPS); m2 = scale * rvar; out = t * m2
            nc.scalar.activation(out=sq_ln[:], in_=sq_var[:], func=ACTF.Ln, bias=eps_n[:])
            nc.scalar.activation(out=sq_rs[:], in_=sq_ln[:], func=ACTF.Exp, scale=-0.5)
            nc.vector.tensor_tensor(out=sq_m2[:], in0=sq_scale[:], in1=sq_rs[:], op=OP.mult)
            nc.vector.tensor_tensor(out=out_sb[:], in0=sq_t[:], in1=bcast(sq_m2[:], D), op=OP.mult)

            if r == R - 1:
                nc.sync.dma_start(out=out_ext[:], in_=out_sb[:])
                break

            # ---------- b update: a[b,i,j] = sum_d out*u_hat ----------
            # outT[d, j, b] via per-j transposes (ACT copy casts to bf16)
            for j in range(J):
                p = pst.tile([16, 128], F32, tag="tr")
                nc.tensor.transpose(p[:], out_sb[:, j, :], ident[:])
                nc.scalar.copy(out=outT[:, j, :], in_=p[:])

            for j in range(J):
                for h in range(4):
                    ps_T = ps_t_pool.tile([128, 2, IL], F32, tag="T")
                    for kk in range(2):
                        k = h * 2 + kk
                        nc.tensor.matmul(
                            ps_T[:, kk, :],
                            lhsT=outT[:, j, :],
                            rhs=w_kd_k[:, k, j, :, :].rearrange("d it i -> d (it i)"),
                            start=True, stop=True,
                        )
                    # ACT moves T out of PSUM (casting to fp16); DVE multiplies
                    nc.scalar.copy(out=t_sb[:, h * 2:(h + 1) * 2, :], in_=ps_T[:])
                nc.vector.tensor_tensor(
                    out=p_big[:], in0=x_kc_bf[:], in1=t_sb[:], op=OP.mult)
                # tree-sum over k -> a[:, :, j] (levels 2+3 on GpSimd)
                nc.vector.tensor_add(tr1[:], p_big[:, :K // 2, :], p_big[:, K // 2:, :])
                nc.gpsimd.tensor_add(tr2[:], tr1[:, :K // 4, :], tr1[:, K // 4:, :])
                nc.gpsimd.tensor_add(a_t[:, :, j], tr2[:, 0, :], tr2[:, 1, :])

            # e = exp(b_logits);   b_logits = sum of a's so far
            if r == 0:
                nc.scalar.activation(out=e_t[:], in_=a_t[:], func=ACTF.Exp)
            else:
                nc.scalar.activation(out=c_t[:], in_=a_t[:], func=ACTF.Exp)
                nc.vector.tensor_tensor(out=e_t[:], in0=e_t[:], in1=c_t[:], op=OP.mult)
            # softmax over j (local): c = e / sum_j e
            nc.vector.reduce_sum(out=z_t[:], in_=e_t[:], axis=AX.X)
            nc.vector.reciprocal_approx_accurate(out=rz_t[:], in_=z_t[:], scratch=rz_scratch[:])
            nc.vector.tensor_tensor(out=c_t[:], in0=e_t[:], in1=bcast(rz_t[:], J), op=OP.mult)
            # cT[(i%128), it, j, b]  (ACT copy casts to bf16)
            for it in range(IT):
                for j in range(J):
                    p = pst.tile([128, 128], F32, tag="tr")
                    nc.tensor.transpose(
                        p[:], c_t[:, it * 128:(it + 1) * 128, j], ident[:])
                    nc.scalar.copy(out=cT_t[:, it, j, :], in_=p[:])

    nc.finalize()
    return nc


_cache = {}


def _get_nc(n_cores: int):
    if n_cores not in _cache:
        _cache[n_cores] = build_nc(n_cores)
    return _cache[n_cores]


def kernel(inputs: np.ndarray, W: np.ndarray) -> np.ndarray:
    assert inputs.shape == (B, I, K) and W.shape == (J, I, D, K)
    IL = I // NCORES
    nc = _get_nc(NCORES)
    in_maps = [
        {
            "x": np.ascontiguousarray(inputs[:, c * IL:(c + 1) * IL, :], dtype=np.float32),
            "w": np.ascontiguousarray(W[:, c * IL:(c + 1) * IL, :, :], dtype=np.float32),
        }
        for c in range(NCORES)
    ]
    res = run_bass_kernel_spmd(nc, in_maps, core_ids=list(range(NCORES)))
    return np.asarray(res.results[0]["out"], dtype=np.float32)


# revision 31
# speedup vs baseline: 1.1383x; 1.1383x over previous
"""Trainium2 Bass kernel for nn_CapsuleLayer (dynamic routing capsule layer).

Reference computation:
    u_hat = einsum('jidk,bik->bjid', W, inputs)        # [B,J,I,D]
    b = 0
    for r in 0..2:
        c = softmax_j(b)                               # [B,J,1,I]
        s = einsum('bjoi,bjid->bjod', c, u_hat)        # [B,J,1,D]
        out = squash(s)
        if r < 2: b += einsum('bjod,bjid->bjoi', out, u_hat)
    return out                                         # [B,J,D]

Strategy: shard I (=2048) across 8 cores (I_loc=256), keep full B=128 on
every core (so PE matmuls stream N=128..256).  u_hat (168 MB) is NEVER
materialized: both routing contractions are expressed against W directly:

    s[b,j,d]  = sum_{i,k} (c[b,j,i] * x[b,i,k]) * W[j,i,d,k]   (PE, K=(k,i))
    a[b,j,i]  = sum_k x[b,i,k] * T[b,j,k,i],
    T[b,j,k,i] = sum_d out[b,j,d] * W[j,i,d,k]                 (PE, K=d)

One 80 KB AllReduce per routing iteration combines the per-core s partial
sums.  Softmax over J is local to each i (no cross-core traffic).

Matmul operands are stored in bf16 (fp32 matmuls on TRN2 cost a 2-way
hi/lo split plus a slow 4-byte LDWEIGHTS; bf16 gets fast-weight-load and
1 cycle/row).  All accumulation stays fp32 (PSUM / DVE internal).
"""

import numpy as np
from contextlib import ExitStack

import concourse.bass as bass
import concourse.bacc as bacc
import concourse.tile as tile
from concourse import mybir
from concourse.bass_utils import run_bass_kernel_spmd
from concourse.masks import make_identity

F32 = mybir.dt.float32
BF16 = mybir.dt.float16  # fp16: 10-bit mantissa, same PE/DVE speed class as bf16
AX = mybir.AxisListType
OP = mybir.AluOpType
ACTF = mybir.ActivationFunctionType

B = 128       # batch
I = 2048      # input capsules (sharded)
K = 8         # DIN
J = 10        # output capsules
D = 16        # DOUT
R = 3         # routing iterations
NCORES = 8

K_EPS = 1e-7
NORM_EPS = 1e-6


def bcast(ap: bass.AP, n: int) -> bass.AP:
    """Append a stride-0 (broadcast) innermost free dim of size n."""
    return bass.AP(ap.tensor, ap.offset, [*ap.ap, [0, n]])


def _pin_activation_tables():
    """Make every activation function we use resolve to the one table set
    that contains them all (natural_log_exp_and_others), so the compiler
    emits a single ACT_TABLE_LOAD instead of thrashing between sets."""
    import concourse.hw_specs as hw_specs

    if getattr(bacc, "_capsule_tables_pinned", False):
        return
    orig = hw_specs.get_activation_tables
    mine = {"Exp", "Ln", "Copy", "Identity", "Square"}

    def patched(module_arch):
        tables = dict(orig(module_arch))
        out = {}
        for name, funcs in tables.items():
            if name == "natural_log_exp_and_others":
                out[name] = funcs
            else:
                out[name] = {f for f in funcs if f.name not in mine}
        return out

    bacc.get_activation_tables = patched
    bacc._capsule_tables_pinned = True


def build_nc(n_cores: int = NCORES):
    IL = I // NCORES          # 256 per-core input capsules (also for n_cores=1 sim)
    IT = IL // 128            # 2 partition tiles of i
    NCH = IL * K // 128       # 16 (k,i)-chunks of 128 contraction rows
    KH = K // 2               # T computed in two 4-k halves (PSUM budget)

    _pin_activation_tables()
    nc = bacc.Bacc(num_devices=n_cores)

    x_ext = nc.dram_tensor("x", [B, IL, K], F32, kind="ExternalInput")
    w_ext = nc.dram_tensor("w", [J, IL, D, K], F32, kind="ExternalInput")
    out_ext = nc.dram_tensor("out", [B, J, D], F32, kind="ExternalOutput")
    ar_in = nc.dram_tensor("ar_in", [B, J * D], F32)
    ar_out = nc.dram_tensor("ar_out", [B, J * D], F32, addr_space="Shared")

    with tile.TileContext(nc) as tc, ExitStack() as ctx:
        sb = ctx.enter_context(tc.tile_pool(name="sb", bufs=1))
        ypool = ctx.enter_context(tc.tile_pool(name="ypool", bufs=2))
        pst = ctx.enter_context(tc.tile_pool(name="pst", bufs=2, space="PSUM"))
        ps_sT_pool = ctx.enter_context(tc.tile_pool(name="ps_sT", bufs=2, space="PSUM"))
        ps_t_pool = ctx.enter_context(tc.tile_pool(name="ps_t", bufs=2, space="PSUM"))
        wpool = ctx.enter_context(tc.tile_pool(name="wpool", bufs=2))

        ident = sb.tile([128, 128], F32)
        make_identity(nc, ident)

        # persistent tensors used throughout
        x_kc = sb.tile([128, K, IL], F32)       # x[b, k, i] (k-outer) fp32
        x_kc_bf = sb.tile([128, K, IL], BF16)   # bf16 copy for the a-phase mul
        x_t = sb.tile([128, NCH, 128], BF16)    # x^T: [(i%128), (k,it), b]
        w_nat = sb.tile([128, IT, J, D * K], F32)   # w[i%128, it, j, (d,k)]
        w_bf = sb.tile([128, IT, J, D * K], BF16)   # bf16 copy (s-matmul rhs)
        w_kd_k = sb.tile([16, K, J, IT, 128], BF16)  # w[d, k, j, it, i] (base-0 rows)
        w_kd_bf = sb.tile([128, J, IT, 128], BF16)   # staging for the re-base

        # ---------------- persistent routing state ----------------
        a_t = sb.tile([128, IL, J], F32)      # agreements (i-outer, j-inner)
        e_t = sb.tile([128, IL, J], F32)      # exp(b_logits)
        z_t = sb.tile([128, IL], F32)         # softmax denominator
        rz_t = sb.tile([128, IL], F32)
        rz_scratch = sb.tile([128, IL], F32)
        c_t = sb.tile([128, IL, J], F32)      # routing weights (reused as exp scratch)
        cT_t = sb.tile([128, IT, J, 128], BF16)  # c transposed: [(i%128), it, j, b]
        s_sb = sb.tile([128, J, D], F32)      # all-reduced s
        s_stage = sb.tile([128, J, D], F32)   # pre-allreduce staging (iter 0)
        sT_stage = sb.tile([16, J, 128], F32)  # pre-allreduce staging (iters 1+)
        sT_sb = sb.tile([16, J, 128], F32)    # all-reduced sT
        out_sb = sb.tile([128, J, D], F32)    # squash output
        outT = sb.tile([16, J, 128], BF16)    # out transposed: [d, j, b]

        # small squash temps
        sq_s2 = sb.tile([128, J], F32)
        sq_mean = sb.tile([128, J], F32)
        sq_t = sb.tile([128, J, D], F32)
        sq_var = sb.tile([128, J], F32)
        sq_ln = sb.tile([128, J], F32)
        sq_rs = sb.tile([128, J], F32)
        sq_u = sb.tile([128, J], F32)
        sq_den = sb.tile([128, J], F32)
        sq_rden = sb.tile([128, J], F32)
        sq_scale = sb.tile([128, J], F32)
        sq_m2 = sb.tile([128, J], F32)
        eps_k = sb.tile([128, 1], F32)
        nc.vector.memset(eps_k[:], K_EPS)
        eps_n = sb.tile([128, 1], F32)
        nc.vector.memset(eps_n[:], NORM_EPS)

        # ---------------- load + relayout ----------------
        # Setup scratch aliases routing-state tensors that are not live yet
        # (a_t / e_t / c_t); Tile's dependency tracking orders the reuse.
        x_nat = bass.AP(a_t.tensor, a_t[:].offset, [a_t[:].ap[0], [K, IL], [1, K]])
        nc.sync.dma_start(out=x_nat, in_=x_ext[:])
        nc.vector.tensor_copy(
            out=x_kc[:],
            in_=bass.AP(a_t.tensor, a_t[:].offset, [a_t[:].ap[0], [1, K], [K, IL]]),
        )
        nc.vector.tensor_copy(out=x_kc_bf[:], in_=x_kc[:])
        for chh in range(NCH):
            k, it = divmod(chh, IT)
            p = pst.tile([128, 128], F32, tag="tr")
            nc.tensor.transpose(p[:], x_kc[:, k, it * 128:(it + 1) * 128], ident[:])
            nc.scalar.copy(out=x_t[:, chh, :], in_=p[:])

        for it in range(IT):
            nc.sync.dma_start(
                out=w_nat[:, it, :, :],
                in_=w_ext.rearrange("j (it p) d k -> it p j (d k)", it=IT)[it],
            )
        nc.vector.tensor_copy(out=w_bf[:], in_=w_nat[:])
        # shuffle (d,k) -> (k,d), transpose to [(k,d), j, it, i], then
        # re-base each k's 16 rows to partition 0 via SBUF->SBUF DMA.
        w_nat2 = bass.AP(
            e_t.tensor, e_t[:].offset,
            [e_t[:].ap[0], [J * K * D, IT], [K * D, J], [D, K], [1, D]])
        nc.vector.tensor_copy(
            out=w_nat2,
            in_=w_nat.rearrange("p it j (d k) -> p it j k d", k=K),
        )
        for it in range(IT):
            for j in range(J):
                p = pst.tile([128, 128], F32, tag="tr")
                nc.tensor.transpose(
                    p[:],
                    bass.AP(e_t.tensor,
                            e_t[:].offset + (it * J + j) * K * D,
                            [e_t[:].ap[0], [1, K * D]]),
                    ident[:],
                )
                nc.scalar.copy(out=w_kd_bf[:, j, it, :], in_=p[:])
        for k in range(K):
            nc.sync.dma_start(
                out=w_kd_k[:, k, :, :, :],
                in_=w_kd_bf[k * 16:(k + 1) * 16, :, :, :],
            )

        def w_rhs(it: int, j: int, k: int) -> bass.AP:
            """bf16 W slice [(i%128) x d] with d strided over the (d,k) dim."""
            return w_bf[:, it, j, :].rearrange("p (d k) -> p k d", k=K)[:, k, :]

        for r in range(R):
            # ---------- s matmuls ----------
            if r == 0:
                # c uniform 1/J: s_raw = sum_i u_hat.  x^T is j-independent,
                # so batch all (j,d) into one N=160 stream per chunk.
                ps_s = ps_sT_pool.tile([128, J, D], F32, tag="sTq")
                for chh in range(NCH):
                    k, it = divmod(chh, IT)
                    rhs_all = w_bf[:, it, :, :].rearrange(
                        "p j (d k) -> p k j d", k=K)[:, k, :, :]
                    nc.tensor.matmul(
                        ps_s[:], lhsT=x_t[:, chh, :],
                        rhs=rhs_all.rearrange("p j d -> p (j d)"),
                        start=(chh == 0), stop=(chh == NCH - 1),
                    )
                nc.scalar.mul(out=s_stage[:], in_=ps_s[:], mul=1.0 / J)
                nc.sync.dma_start(
                    out=ar_in[:], in_=s_stage.rearrange("b j d -> b (j d)"))
            else:
                # stationary = W slices (16-col weight loads); moving = Y
                # (N=128).  Four j's run concurrently in separate 32-column
                # groups of the PE array (tile_position col-tiling), so the
                # per-matmul weight-load latency overlaps.  Output lands
                # transposed (sT[d, b] per j) and is all-reduced in that
                # layout (elementwise add is layout-agnostic); only the
                # reduced result is transposed back.
                for q in range((J + 3) // 4):
                    js = list(range(4 * q, min(4 * q + 4, J)))
                    y_q = ypool.tile([128, NCH, 4, 128], BF16, tag="y")
                    for g, j in enumerate(js):
                        cT_b = bass.AP(
                            cT_t.tensor, cT_t[:, 0, j, :].offset,
                            [cT_t.ap[0], [0, K], cT_t.ap[1], cT_t.ap[3]],
                        )  # dims [p, k(bcast), it, b]
                        nc.vector.tensor_tensor(
                            out=y_q[:, :, g, :], in0=x_t[:], in1=cT_b, op=OP.mult,
                        )
                    ps_q = ps_sT_pool.tile([128, 128], F32, tag="sTq")
                    for chh in range(NCH):
                        k, it = divmod(chh, IT)
                        for g, j in enumerate(js):
                            nc.tensor.matmul(
                                ps_q[32 * g:32 * g + 16, :],
                                lhsT=w_rhs(it, j, k), rhs=y_q[:, chh, g, :],
                                start=(chh == 0), stop=(chh == NCH - 1),
                                tile_position=(0, 32 * g),
                                skip_group_check=True,
                            )
                    for g, j in enumerate(js):
                        nc.scalar.copy(
                            out=sT_stage[:, j, :], in_=ps_q[32 * g:32 * g + 16, :])
                nc.sync.dma_start(
                    out=ar_in.rearrange("b f -> (b f)").rearrange(
                        "(p f) -> p f", p=16),
                    in_=sT_stage.rearrange("d j b -> d (j b)"))

            # ---------- all-reduce s ----------
            if n_cores > 1:
                nc.gpsimd.collective_compute(
                    "AllReduce", OP.add,
                    replica_groups=[list(range(n_cores))],
                    ins=[ar_in[:]], outs=[ar_out[:]],
                )
                ar_res = ar_out
            else:
                ar_res = ar_in

            if r == 0:
                nc.sync.dma_start(
                    out=s_sb.rearrange("b j d -> b (j d)"), in_=ar_res[:])
            else:
                nc.sync.dma_start(
                    out=sT_sb.rearrange("d j b -> d (j b)"),
                    in_=ar_res.rearrange("b f -> (b f)").rearrange(
                        "(p f) -> p f", p=16))
                for j in range(J):
                    p = pst.tile([128, 128], F32, tag="tr")
                    nc.tensor.transpose(
                        p[:, :16], sT_sb[:, j, :], ident[:16, :16])
                    nc.scalar.copy(out=s_sb[:, j, :], in_=p[:, :16])

            # ---------- squash ----------
            v = s_sb  # [128, J, D]
            # s2 = sum_d (v/5)^2 ; mean = sum_d v / D
            nc.vector.scalar_tensor_tensor(
                out=sq_t[:], in0=v[:], scalar=0.04, in1=v[:],
                op0=OP.mult, op1=OP.mult)
            nc.vector.reduce_sum(out=sq_s2[:], in_=sq_t[:], axis=AX.X)
            nc.vector.reduce_sum(out=sq_mean[:], in_=v[:], axis=AX.X)
            nc.vector.tensor_scalar_mul(sq_mean[:], sq_mean[:], 1.0 / D)
            # t = v - mean ; var = sum_d t^2 / D
            nc.vector.tensor_sub(sq_t[:], v[:], bcast(sq_mean[:], D))
            nc.vector.tensor_tensor(out=v[:], in0=sq_t[:], in1=sq_t[:], op=OP.mult)
            nc.vector.reduce_sum(out=sq_var[:], in_=v[:], axis=AX.X)
            nc.vector.tensor_scalar_mul(sq_var[:], sq_var[:], 1.0 / D)
            # rs = 1/sqrt(s2 + K_EPS) = exp(-0.5*ln(s2 + K_EPS))
            nc.scalar.activation(out=sq_ln[:], in_=sq_s2[:], func=ACTF.Ln, bias=eps_k[:])
            nc.scalar.activation(out=sq_rs[:], in_=sq_ln[:], func=ACTF.Exp, scale=-0.5)
            # scale = 0.5*s2/(1+0.5*s2) * rs
            nc.vector.tensor_scalar_mul(sq_u[:], sq_s2[:], 0.5)
            nc.vector.tensor_scalar_add(sq_den[:], sq_u[:], 1.0)
            nc.vector.reciprocal(out=sq_rden[:], in_=sq_den[:])
            nc.vector.tensor_tensor(out=sq_scale[:], in0=sq_u[:], in1=sq_rden[:], op=OP.mult)
            nc.vector.tensor_tensor(out=sq_scale[:], in0=sq_scale[:], in1=sq_rs[:], op=OP.mult)
            # rvar = 1/sqrt(var + NORM_EPS); m2 = scale * rvar; out = t * m2
            nc.scalar.activation(out=sq_ln[:], in_=sq_var[:], func=ACTF.Ln, bias=eps_n[:])
            nc.scalar.activation(out=sq_rs[:], in_=sq_ln[:], func=ACTF.Exp, scale=-0.5)
            nc.vector.tensor_tensor(out=sq_m2[:], in0=sq_scale[:], in1=sq_rs[:], op=OP.mult)
            nc.vector.tensor_tensor(out=out_sb[:], in0=sq_t[:], in1=bcast(sq_m2[:], D), op=OP.mult)

            if r == R - 1:
                nc.sync.dma_start(out=out_ext[:], in_=out_sb[:])
                break

            # ---------- b update: a[b,i,j] = sum_d out*u_hat ----------
            # outT[d, j, b] via per-j transposes (ACT copy casts to bf16)
            for j in range(J):
                p = pst.tile([16, 128], F32, tag="tr")
                nc.tensor.transpose(p[:], out_sb[:, j, :], ident[:])
                nc.scalar.copy(out=outT[:, j, :], in_=p[:])

            for j in range(J):
                t_sb = wpool.tile([128, K, IL], BF16, tag="t_sb")
                for h in range(2):
                    ps_T = ps_t_pool.tile([128, KH, IL], F32, tag="T")
                    for kk in range(KH):
                        k = h * KH + kk
                        nc.tensor.matmul(
                            ps_T[:, kk, :],
                            lhsT=outT[:, j, :],
                            rhs=w_kd_k[:, k, j, :, :].rearrange("d it i -> d (it i)"),
                            start=True, stop=True,
                        )
                    # ACT moves T out of PSUM (casting to fp16); DVE multiplies
                    nc.scalar.copy(out=t_sb[:, h * KH:(h + 1) * KH, :], in_=ps_T[:])
                p_big = wpool.tile([128, K, IL], BF16, tag="p_big")
                tr1 = wpool.tile([128, K // 2, IL], BF16, tag="tr1")
                tr2 = wpool.tile([128, K // 4, IL], BF16, tag="tr2")
                nc.vector.tensor_tensor(
                    out=p_big[:], in0=x_kc_bf[:], in1=t_sb[:], op=OP.mult)
                # tree-sum over k -> a[:, :, j]
                nc.vector.tensor_add(tr1[:], p_big[:, :K // 2, :], p_big[:, K // 2:, :])
                nc.vector.tensor_add(tr2[:], tr1[:, :K // 4, :], tr1[:, K // 4:, :])
                nc.vector.tensor_add(a_t[:, :, j], tr2[:, 0, :], tr2[:, 1, :])

            # e = exp(b_logits);   b_logits = sum of a's so far
            if r == 0:
                nc.scalar.activation(out=e_t[:], in_=a_t[:], func=ACTF.Exp)
            else:
                nc.scalar.activation(out=c_t[:], in_=a_t[:], func=ACTF.Exp)
                nc.vector.tensor_tensor(out=e_t[:], in0=e_t[:], in1=c_t[:], op=OP.mult)
            # softmax over j (local): c = e / sum_j e
            nc.vector.reduce_sum(out=z_t[:], in_=e_t[:], axis=AX.X)
            nc.vector.reciprocal_approx_accurate(out=rz_t[:], in_=z_t[:], scratch=rz_scratch[:])
            nc.vector.tensor_tensor(out=c_t[:], in0=e_t[:], in1=bcast(rz_t[:], J), op=OP.mult)
            # cT[(i%128), it, j, b]  (ACT copy casts to bf16)
            for it in range(IT):
                for j in range(J):
                    p = pst.tile([128, 128], F32, tag="tr")
                    nc.tensor.transpose(
                        p[:], c_t[:, it * 128:(it + 1) * 128, j], ident[:])
                    nc.scalar.copy(out=cT_t[:, it, j, :], in_=p[:])

    nc.finalize()
    return nc


_cache = {}


def _get_nc(n_cores: int):
    if n_cores not in _cache:
        _cache[n_cores] = build_nc(n_cores)
    return _cache[n_cores]


def kernel(inputs: np.ndarray, W: np.ndarray) -> np.ndarray:
    assert inputs.shape == (B, I, K) and W.shape == (J, I, D, K)
    IL = I // NCORES
    nc = _get_nc(NCORES)
    in_maps = [
        {
            "x": np.ascontiguousarray(inputs[:, c * IL:(c + 1) * IL, :], dtype=np.float32),
            "w": np.ascontiguousarray(W[:, c * IL:(c + 1) * IL, :, :], dtype=np.float32),
        }
        for c in range(NCORES)
    ]
    res = run_bass_kernel_spmd(nc, in_maps, core_ids=list(range(NCORES)))
    return np.asarray(res.results[0]["out"], dtype=np.float32)
